# revision 20
# baseline (speedup 1.0000x reference)
"""Self-contained Bass/Trainium2 kernel for nn_Attention (B=4, N=2048, D=1024, H=16, dh=64).

Sharding: 8 cores = (batch b in 0..3) x (head-group g in 0..1, 8 heads each).
Each core projects q/k/v only for its own 8 heads over the full sequence
(no duplicated projection work), runs attention for those heads over all
2048 rows, and computes a partial output projection against its 512 rows
of w_out. The host sums the two partial outputs per batch (row-parallel
linear reduction) -- no on-chip communication.

Numerics: projections in float32r (TF32-class), q/k/v and attention
weights in fp16, accumulation in fp32 PSUM. Softmax uses exp-sum-divide
without max subtraction (scores are O(1)); row sums come free from a
ones-column appended to V; 1/sum via the fast approximate DVE reciprocal.
The bias is added on even cores only (odd cores receive zeros) so the
host-side pair sum stays correct.
"""

import sys
import numpy as np

sys.path.insert(0, "/opt/trn_rl_repo")

B, N, DIM = 4, 2048, 1024
HEADS, DH = 16, 64
SCALE = DH ** -0.5  # 0.125
NC = 8
HLOC = HEADS // 2        # 8 heads per core
GD = HLOC * DH           # 512 projected dims per core per q/k/v
HP = HLOC // 2           # 4 head pairs
CT = DIM // 128          # 8 contraction tiles over input channels
JT = N // 128            # 16 key tiles
ICH = 512                # i-chunk width for attention
NCH = N // ICH           # 4 chunks
VW = DH + 1              # 65: v block width incl. ones column

_compiled = None


def _build():
    import concourse.tile as tile
    from concourse import bacc, mybir

    f32 = mybir.dt.float32
    f32r = mybir.dt.float32r
    f16 = mybir.dt.float16
    EXP = mybir.ActivationFunctionType.Exp

    nc = bacc.Bacc("TRN2", target_bir_lowering=False, debug=False, num_devices=NC)

    X = nc.dram_tensor("x", (DIM, N), f32r, kind="ExternalInput").ap()
    WQKV = nc.dram_tensor("w_qkv", (DIM, 3 * GD), f32r, kind="ExternalInput").ap()
    WOUT = nc.dram_tensor("w_out", (GD, DIM), f32r, kind="ExternalInput").ap()
    BOUT = nc.dram_tensor("b_out", (DIM,), f32, kind="ExternalInput").ap()
    Y = nc.dram_tensor("y", (N, DIM), f32, kind="ExternalOutput").ap()

    with tile.TileContext(nc) as tc:
        with tc.tile_pool(name="persist", bufs=1) as persist, \
             tc.tile_pool(name="attn", bufs=1) as attn:

            # ---- persistent SBUF tensors ----
            w_sb = [persist.tile([128, 3 * GD], f32r, tag="wsb", bufs=CT,
                                 name=f"wsb{ct}") for ct in range(CT)]
            wo_sb = [persist.tile([128, DIM], f32r, tag="wo", bufs=HP,
                                  name=f"wo{hp}") for hp in range(HP)]
            qT = [persist.tile([128, N], f16, tag="qT", bufs=HP,
                              name=f"qT{m}") for m in range(HP)]
            kT = [persist.tile([128, N], f16, tag="kT", bufs=HP,
                              name=f"kT{m}") for m in range(HP)]
            v_ext = [persist.tile([128, HLOC * VW], f16, tag="vext", bufs=JT,
                                  name=f"vext{t}") for t in range(JT)]

            bias_src = persist.tile([1, DIM], f32, tag="bias_src")
            nc.sync.dma_start(bias_src[:], BOUT.rearrange("(o d) -> o d", o=1))
            bias = persist.tile([128, DIM], f32, tag="bias")
            nc.gpsimd.partition_broadcast(bias[:], bias_src[0:1, :])

            # ones columns of v_ext don't depend on data: set them up front
            for t in range(JT):
                ones_col = v_ext[t].rearrange("p (hh c) -> p hh c", c=VW)[:, :, DH:VW]
                nc.gpsimd.memset(ones_col, 1.0)

            # ================= phase A: q/k/v projections =================
            with tc.tile_pool(name="stage", bufs=1) as stage:
                # ---- input DMA, ordered to match first-use ----
                x_sb = {}

                def load_x(c):
                    x_sb[c] = [None] * CT
                    for ct in range(CT):
                        t = stage.tile([128, ICH], f32r, tag="xsb", bufs=2 * CT,
                                       name=f"x{c}_{ct}")
                        nc.sync.dma_start(
                            t[:], X[ct * 128:(ct + 1) * 128, c * ICH:(c + 1) * ICH])
                        x_sb[c][ct] = t

                def load_w(third, ct):
                    nc.sync.dma_start(
                        w_sb[ct][:, third * GD:(third + 1) * GD],
                        WQKV[ct * 128:(ct + 1) * 128, third * GD:(third + 1) * GD])

                # interleave x chunk 0 with w-q so the first accumulation
                # group streams as its operands land
                x_sb[0] = [None] * CT
                for ct in range(CT):
                    t = stage.tile([128, ICH], f32r, tag="xsb", bufs=2 * CT,
                                   name=f"x0_{ct}")
                    nc.sync.dma_start(t[:], X[ct * 128:(ct + 1) * 128, 0:ICH])
                    x_sb[0][ct] = t
                    load_w(0, ct)
                for ct in range(CT):
                    load_w(1, ct)
                for ct in range(CT):
                    load_w(2, ct)
                load_x(1)

                with tc.tile_pool(name="psA", bufs=1, space="PSUM") as psA:
                    for c in range(NCH):
                        xs = x_sb[c]
                        csl = slice(c * ICH, (c + 1) * ICH)
                        # q then k: psum [128 dims, 512 cols], contract channels
                        for third, dst in ((0, qT), (1, kT)):
                            for m in range(HP):
                                ps = psA.tile([128, ICH], f32, tag="proj", bufs=4,
                                              name=f"ps{third}_{c}_{m}")
                                base = third * GD + m * 128
                                for ct in range(CT):
                                    nc.tensor.matmul(ps[:], w_sb[ct][:, base:base + 128],
                                                     xs[ct][:],
                                                     start=(ct == 0), stop=(ct == CT - 1))
                                nc.vector.tensor_copy(dst[m][:, csl], ps[:])
                        # v directly in [row, dim] layout: x block is stationary
                        for jl in range(ICH // 128):
                            ps = psA.tile([128, GD], f32, tag="proj", bufs=4,
                                          name=f"psv_{c}_{jl}")
                            for ct in range(CT):
                                nc.tensor.matmul(ps[:],
                                                 xs[ct][:, jl * 128:(jl + 1) * 128],
                                                 w_sb[ct][:, 2 * GD:3 * GD],
                                                 start=(ct == 0), stop=(ct == CT - 1))
                            dst = v_ext[c * (ICH // 128) + jl].rearrange(
                                "p (hh c) -> p hh c", c=VW)[:, :, 0:DH]
                            nc.vector.tensor_copy(
                                dst, ps.rearrange("p (hh c) -> p hh c", c=DH))
                        # prefetch x for chunk c+2 only after chunk c's reads
                        # are emitted (slot-reuse WAR dependency needs them)
                        if c + 2 < NCH:
                            load_x(c + 2)
                        if c == 0:  # w_out needed only in phase B
                            for hp in range(HP):
                                nc.sync.dma_start(wo_sb[hp][:],
                                                  WOUT[hp * 128:(hp + 1) * 128, :])

            # ================= phase B: attention + output projection =================
            with tc.tile_pool(name="psB", bufs=1, space="PSUM") as psB:
                ctx = {}

                def make_out_ops(ch):
                    """32 micro-ops computing y rows [ch*512, (ch+1)*512)."""
                    ops = []
                    ypb = {}

                    def mk(ib, ec, hp):
                        def op():
                            if hp == 0:
                                ypb[(ib, ec)] = psB.tile(
                                    [128, 512], f32, tag="yp", bufs=2,
                                    name=f"yp{ch}_{ib}_{ec}")
                            yp = ypb[(ib, ec)]
                            nc.tensor.matmul(
                                yp[:], ctx[(ch, hp)][ib][:, :],
                                wo_sb[hp][:, ec * 512:(ec + 1) * 512],
                                start=(hp == 0), stop=(hp == HP - 1))
                            if hp == HP - 1:
                                ysb = attn.tile([128, 512], f32, tag="ysb", bufs=2,
                                                name=f"ysb{ch}_{ib}_{ec}")
                                nc.vector.tensor_add(
                                    ysb[:], yp[:], bias[:, ec * 512:(ec + 1) * 512])
                                r0 = ch * ICH + ib * 128
                                nc.sync.dma_start(
                                    Y[r0:r0 + 128, ec * 512:(ec + 1) * 512], ysb[:])
                        return op

                    for ib in range(ICH // 128):
                        for ec in range(2):
                            for hp in range(HP):
                                ops.append(mk(ib, ec, hp))
                    return ops

                for ch in range(NCH):
                    isl = slice(ch * ICH, (ch + 1) * ICH)
                    oplist = make_out_ops(ch - 1) if ch > 0 else []
                    k = 0
                    for hp in range(HP):
                        # one tile per 128-col block: the out-proj reads then
                        # depend only on their own block's normalize mul
                        cx = [attn.tile([128, 128], f32r, tag="ctx", bufs=32,
                                        name=f"ctx{ch}_{hp}_{b4}")
                              for b4 in range(4)]
                        ctx[(ch, hp)] = cx
                        po = [psB.tile([65, ICH], f32, tag="po", bufs=2,
                                       name=f"po{ch}_{hp}_{p}") for p in range(2)]
                        ats = {}

                        def av(j):
                            for p in range(2):
                                hd = 2 * hp + p
                                nc.tensor.matmul(
                                    po[p][:],
                                    v_ext[j][:, hd * VW:(hd + 1) * VW],
                                    ats[j][:, p * 512:(p + 1) * 512],
                                    start=(j == 0), stop=(j == JT - 1))

                        for jt in range(JT):
                            pp = psB.tile([128, 1024], f32, tag="dots", bufs=2,
                                          name=f"pp{ch}_{hp}_{jt}")
                            for p in range(2):
                                nc.tensor.matmul(
                                    pp[:, p * 512:(p + 1) * 512],
                                    kT[hp][p * 64:(p + 1) * 64, jt * 128:(jt + 1) * 128],
                                    qT[hp][p * 64:(p + 1) * 64, isl],
                                    start=True, stop=True)
                            at = attn.tile([128, 1024], f16, tag="at", bufs=5,
                                           name=f"at{ch}_{hp}_{jt}")
                            nc.scalar.activation(at[:], pp[:], EXP,
                                                 bias=0.0, scale=SCALE)
                            ats[jt] = at
                            # lag-2 so PE never head-of-line blocks on ACT jitter
                            if jt >= 2:
                                av(jt - 2)
                                del ats[jt - 2]
                            # interleave out-proj only in the hp's back half so
                            # it never waits on the previous chunk's last
                            # normalize chain (still in the DVE pipe early on)
                            if jt >= JT - 8 and k < len(oplist):
                                oplist[k]()
                                k += 1
                        av(JT - 2)
                        av(JT - 1)
                        # evacuate po at once (a single cheap copy frees the
                        # PSUM slot); the slow reciprocal chain then runs off
                        # the SBUF copy without gating the next head-pair
                        cxu = []
                        for p in range(2):
                            cu = attn.tile([65, ICH], f32, tag="cxu", bufs=4,
                                           name=f"cxu{ch}_{hp}_{p}")
                            nc.vector.tensor_copy(cu[:], po[p][:])
                            cxu.append(cu)
                        # normalize in 128-col blocks: DVE op time scales with
                        # free size, so the first ctx block is ready ~3x sooner
                        # and the scheduler-hoisted out-proj reads barely wait
                        for b4 in range(4):
                            bsl = slice(b4 * 128, (b4 + 1) * 128)
                            rss = []
                            for p in range(2):
                                rs = attn.tile([1, 128], f32, tag="rs", bufs=4,
                                               name=f"rs{ch}_{hp}_{p}_{b4}")
                                nc.vector.reciprocal(rs[:], cxu[p][64:65, bsl])
                                rss.append(rs)
                            for p in range(2):
                                rb = attn.tile([128, 128], f32, tag="rb", bufs=4,
                                               name=f"rb{ch}_{hp}_{p}_{b4}")
                                nc.gpsimd.partition_broadcast(rb[:], rss[p][0:1, :])
                                nc.vector.tensor_mul(cx[b4][p * 64:(p + 1) * 64, :],
                                                     cxu[p][0:64, bsl],
                                                     rb[0:64, :])
                    while k < len(oplist):
                        oplist[k]()
                        k += 1
                # tail: output projection of the last chunk
                for op in make_out_ops(NCH - 1):
                    op()

    nc.compile()
    return nc


def _get_compiled():
    global _compiled
    if _compiled is None:
        _compiled = _build()
    return _compiled


def _make_in_maps(x, w_qkv, w_out, b_out):
    x = np.asarray(x, dtype=np.float32)
    w_qkv = np.asarray(w_qkv, dtype=np.float32)
    w_out = np.asarray(w_out, dtype=np.float32)
    b_out = np.asarray(b_out, dtype=np.float32)
    zeros = np.zeros_like(b_out)

    xT = [np.ascontiguousarray(x[b].T) for b in range(B)]
    wq = []
    wo = []
    for g in range(2):
        cols = np.concatenate(
            [w_qkv[:, t * DIM + g * GD: t * DIM + (g + 1) * GD] for t in range(3)],
            axis=1)
        wq.append(np.ascontiguousarray(cols))
        wo.append(np.ascontiguousarray(w_out[g * GD:(g + 1) * GD]))

    in_maps = []
    for c in range(NC):
        b, g = divmod(c, 2)
        in_maps.append({"x": xT[b], "w_qkv": wq[g], "w_out": wo[g],
                        "b_out": b_out if g == 0 else zeros})
    return in_maps


def kernel(x, w_qkv, w_out, b_out):
    from concourse.bass_utils import run_bass_kernel_spmd

    nc = _get_compiled()
    in_maps = _make_in_maps(x, w_qkv, w_out, b_out)
    res = run_bass_kernel_spmd(nc, in_maps, core_ids=list(range(NC)))

    out = np.empty((B, N, DIM), dtype=np.float32)
    for b in range(B):
        out[b] = res.results[2 * b]["y"] + res.results[2 * b + 1]["y"]
    return out


# revision 24
# speedup vs baseline: 1.1817x; 1.1817x over previous
"""Self-contained Bass/Trainium2 kernel for nn_Attention (B=4, N=2048, D=1024, H=16, dh=64).

Sharding: 8 cores = (batch b in 0..3) x (head-group g in 0..1, 8 heads each).
Each core projects q/k/v only for its own 8 heads over the full sequence
(no duplicated projection work), runs attention for those heads over all
2048 rows, and computes a partial output projection against its 512 rows
of w_out. The host sums the two partial outputs per batch (row-parallel
linear reduction) -- no on-chip communication.

Numerics: projections in float32r (TF32-class), q/k/v and attention
weights in fp16, accumulation in fp32 PSUM. Softmax uses exp-sum-divide
without max subtraction (scores are O(1)); row sums come free from a
ones-column appended to V; 1/sum via the fast approximate DVE reciprocal.
The bias is added on even cores only (odd cores receive zeros) so the
host-side pair sum stays correct.
"""

import sys
import numpy as np

sys.path.insert(0, "/opt/trn_rl_repo")

B, N, DIM = 4, 2048, 1024
HEADS, DH = 16, 64
SCALE = DH ** -0.5  # 0.125
NC = 8
HLOC = HEADS // 2        # 8 heads per core
GD = HLOC * DH           # 512 projected dims per core per q/k/v
HP = HLOC // 2           # 4 head pairs
CT = DIM // 128          # 8 contraction tiles over input channels
JT = N // 128            # 16 key tiles
ICH = 512                # i-chunk width for attention
NCH = N // ICH           # 4 chunks
VW = DH + 1              # 65: v block width incl. ones column

_compiled = None


def _build():
    import concourse.tile as tile
    from concourse import bacc, mybir

    f32 = mybir.dt.float32
    f32r = mybir.dt.float32r
    f16 = mybir.dt.float16
    EXP = mybir.ActivationFunctionType.Exp

    nc = bacc.Bacc("TRN2", target_bir_lowering=False, debug=False, num_devices=NC)

    X = nc.dram_tensor("x", (DIM, N), f32r, kind="ExternalInput").ap()
    WQKV = nc.dram_tensor("w_qkv", (DIM, 3 * GD), f32r, kind="ExternalInput").ap()
    WOUT = nc.dram_tensor("w_out", (GD, DIM), f32r, kind="ExternalInput").ap()
    BOUT = nc.dram_tensor("b_out", (DIM,), f32, kind="ExternalInput").ap()
    Y = nc.dram_tensor("y", (N, DIM), f32, kind="ExternalOutput").ap()

    with tile.TileContext(nc) as tc:
        with tc.tile_pool(name="persist", bufs=1) as persist, \
             tc.tile_pool(name="attn", bufs=1) as attn:

            # ---- persistent SBUF tensors ----
            w_sb = [persist.tile([128, 3 * GD], f32r, tag="wsb", bufs=CT,
                                 name=f"wsb{ct}") for ct in range(CT)]
            wo_sb = [persist.tile([128, DIM], f32r, tag="wo", bufs=HP,
                                  name=f"wo{hp}") for hp in range(HP)]
            qT = [persist.tile([128, N], f16, tag="qT", bufs=HP,
                              name=f"qT{m}") for m in range(HP)]
            kT = [persist.tile([128, N], f16, tag="kT", bufs=HP,
                              name=f"kT{m}") for m in range(HP)]
            v_ext = [persist.tile([128, HLOC * VW], f16, tag="vext", bufs=JT,
                                  name=f"vext{t}") for t in range(JT)]

            bias_src = persist.tile([1, DIM], f32, tag="bias_src")
            nc.sync.dma_start(bias_src[:], BOUT.rearrange("(o d) -> o d", o=1))
            bias = persist.tile([128, DIM], f32, tag="bias")
            nc.gpsimd.partition_broadcast(bias[:], bias_src[0:1, :])

            # ones columns of v_ext don't depend on data: set them up front
            for t in range(JT):
                ones_col = v_ext[t].rearrange("p (hh c) -> p hh c", c=VW)[:, :, DH:VW]
                nc.gpsimd.memset(ones_col, 1.0)

            # ================= phase A: q/k/v projections =================
            with tc.tile_pool(name="stage", bufs=1) as stage:
                # ---- input DMA, ordered to match first-use ----
                x_sb = {}

                def load_x(c):
                    x_sb[c] = [None] * CT
                    for ct in range(CT):
                        t = stage.tile([128, ICH], f32r, tag="xsb", bufs=2 * CT,
                                       name=f"x{c}_{ct}")
                        nc.sync.dma_start(
                            t[:], X[ct * 128:(ct + 1) * 128, c * ICH:(c + 1) * ICH])
                        x_sb[c][ct] = t

                def load_w(third, ct):
                    nc.sync.dma_start(
                        w_sb[ct][:, third * GD:(third + 1) * GD],
                        WQKV[ct * 128:(ct + 1) * 128, third * GD:(third + 1) * GD])

                # interleave x chunk 0 with w-q so the first accumulation
                # group streams as its operands land
                x_sb[0] = [None] * CT
                for ct in range(CT):
                    t = stage.tile([128, ICH], f32r, tag="xsb", bufs=2 * CT,
                                   name=f"x0_{ct}")
                    nc.sync.dma_start(t[:], X[ct * 128:(ct + 1) * 128, 0:ICH])
                    x_sb[0][ct] = t
                    load_w(0, ct)
                for ct in range(CT):
                    load_w(1, ct)
                for ct in range(CT):
                    load_w(2, ct)
                load_x(1)

                with tc.tile_pool(name="psA", bufs=1, space="PSUM") as psA:
                    for c in range(NCH):
                        xs = x_sb[c]
                        csl = slice(c * ICH, (c + 1) * ICH)
                        # q then k: psum [128 dims, 512 cols], contract channels
                        for third, dst in ((0, qT), (1, kT)):
                            for m in range(HP):
                                ps = psA.tile([128, ICH], f32, tag="proj", bufs=4,
                                              name=f"ps{third}_{c}_{m}")
                                base = third * GD + m * 128
                                for ct in range(CT):
                                    nc.tensor.matmul(ps[:], w_sb[ct][:, base:base + 128],
                                                     xs[ct][:],
                                                     start=(ct == 0), stop=(ct == CT - 1))
                                nc.vector.tensor_copy(dst[m][:, csl], ps[:])
                        # v directly in [row, dim] layout: x block is stationary
                        for jl in range(ICH // 128):
                            ps = psA.tile([128, GD], f32, tag="proj", bufs=4,
                                          name=f"psv_{c}_{jl}")
                            for ct in range(CT):
                                nc.tensor.matmul(ps[:],
                                                 xs[ct][:, jl * 128:(jl + 1) * 128],
                                                 w_sb[ct][:, 2 * GD:3 * GD],
                                                 start=(ct == 0), stop=(ct == CT - 1))
                            dst = v_ext[c * (ICH // 128) + jl].rearrange(
                                "p (hh c) -> p hh c", c=VW)[:, :, 0:DH]
                            nc.vector.tensor_copy(
                                dst, ps.rearrange("p (hh c) -> p hh c", c=DH))
                        # prefetch x for chunk c+2 only after chunk c's reads
                        # are emitted (slot-reuse WAR dependency needs them)
                        if c + 2 < NCH:
                            load_x(c + 2)
                        if c == 0:  # w_out needed only in phase B
                            for hp in range(HP):
                                nc.sync.dma_start(wo_sb[hp][:],
                                                  WOUT[hp * 128:(hp + 1) * 128, :])

            # ================= phase B: attention + output projection =================
            with tc.tile_pool(name="psB", bufs=1, space="PSUM") as psB:
                ctx = {}

                def make_out_ops(ch):
                    """32 micro-ops computing y rows [ch*512, (ch+1)*512)."""
                    ops = []
                    ypb = {}

                    def mk(ib, ec, hp):
                        def op():
                            if hp == 0:
                                ypb[(ib, ec)] = psB.tile(
                                    [128, 512], f32, tag="yp", bufs=2,
                                    name=f"yp{ch}_{ib}_{ec}")
                            yp = ypb[(ib, ec)]
                            nc.tensor.matmul(
                                yp[:], ctx[(ch, hp)][:, ib * 128:(ib + 1) * 128],
                                wo_sb[hp][:, ec * 512:(ec + 1) * 512],
                                start=(hp == 0), stop=(hp == HP - 1))
                            if hp == HP - 1:
                                ysb = attn.tile([128, 512], f32, tag="ysb", bufs=2,
                                                name=f"ysb{ch}_{ib}_{ec}")
                                nc.vector.tensor_add(
                                    ysb[:], yp[:], bias[:, ec * 512:(ec + 1) * 512])
                                r0 = ch * ICH + ib * 128
                                nc.sync.dma_start(
                                    Y[r0:r0 + 128, ec * 512:(ec + 1) * 512], ysb[:])
                        return op

                    for ib in range(ICH // 128):
                        for ec in range(2):
                            for hp in range(HP):
                                ops.append(mk(ib, ec, hp))
                    return ops

                for ch in range(NCH):
                    isl = slice(ch * ICH, (ch + 1) * ICH)
                    oplist = make_out_ops(ch - 1) if ch > 0 else []
                    k = 0
                    for hp in range(HP):
                        cx = attn.tile([128, ICH], f32r, tag="ctx", bufs=8,
                                       name=f"ctx{ch}_{hp}")
                        ctx[(ch, hp)] = cx
                        po = [psB.tile([65, ICH], f32, tag="po", bufs=2,
                                       name=f"po{ch}_{hp}_{p}") for p in range(2)]
                        ats = {}

                        def av(j):
                            for p in range(2):
                                hd = 2 * hp + p
                                nc.tensor.matmul(
                                    po[p][:],
                                    v_ext[j][:, hd * VW:(hd + 1) * VW],
                                    ats[j][:, p * 512:(p + 1) * 512],
                                    start=(j == 0), stop=(j == JT - 1))

                        for jt in range(JT):
                            pp = psB.tile([128, 1024], f32, tag="dots", bufs=2,
                                          name=f"pp{ch}_{hp}_{jt}")
                            for p in range(2):
                                nc.tensor.matmul(
                                    pp[:, p * 512:(p + 1) * 512],
                                    kT[hp][p * 64:(p + 1) * 64, jt * 128:(jt + 1) * 128],
                                    qT[hp][p * 64:(p + 1) * 64, isl],
                                    start=True, stop=True)
                            at = attn.tile([128, 1024], f16, tag="at", bufs=5,
                                           name=f"at{ch}_{hp}_{jt}")
                            nc.scalar.activation(at[:], pp[:], EXP,
                                                 bias=0.0, scale=SCALE)
                            ats[jt] = at
                            # lag-2 so PE never head-of-line blocks on ACT jitter
                            if jt >= 2:
                                av(jt - 2)
                                del ats[jt - 2]
                            # interleave out-proj only in the hp's back half so
                            # it never waits on the previous chunk's last
                            # normalize chain (still in the DVE pipe early on)
                            if jt >= JT - 8 and k < len(oplist):
                                # bias the scheduler against hoisting these to
                                # the chunk boundary where ctx is still in the
                                # normalize pipe
                                tc.cur_priority += 24
                                oplist[k]()
                                tc.cur_priority -= 24
                                k += 1
                        av(JT - 2)
                        av(JT - 1)
                        # evacuate po at once (a single cheap copy frees the
                        # PSUM slot); the slow reciprocal chain then runs off
                        # the SBUF copy without gating the next head-pair
                        cxu = []
                        for p in range(2):
                            cu = attn.tile([65, ICH], f32, tag="cxu", bufs=4,
                                           name=f"cxu{ch}_{hp}_{p}")
                            nc.vector.tensor_copy(cu[:], po[p][:])
                            cxu.append(cu)
                        # normalize in 128-col blocks: DVE op time scales with
                        # free size, so the first ctx block is ready ~3x sooner
                        # and the scheduler-hoisted out-proj reads barely wait
                        for b4 in range(4):
                            bsl = slice(b4 * 128, (b4 + 1) * 128)
                            rss = []
                            for p in range(2):
                                rs = attn.tile([1, 128], f32, tag="rs", bufs=4,
                                               name=f"rs{ch}_{hp}_{p}_{b4}")
                                nc.vector.reciprocal(rs[:], cxu[p][64:65, bsl])
                                rss.append(rs)
                            for p in range(2):
                                rb = attn.tile([128, 128], f32, tag="rb", bufs=4,
                                               name=f"rb{ch}_{hp}_{p}_{b4}")
                                nc.gpsimd.partition_broadcast(rb[:], rss[p][0:1, :])
                                nc.vector.tensor_mul(cx[p * 64:(p + 1) * 64, bsl],
                                                     cxu[p][0:64, bsl],
                                                     rb[0:64, :])
                    while k < len(oplist):
                        oplist[k]()
                        k += 1
                # tail: output projection of the last chunk
                for op in make_out_ops(NCH - 1):
                    op()

    nc.compile()
    return nc


def _get_compiled():
    global _compiled
    if _compiled is None:
        _compiled = _build()
    return _compiled


def _make_in_maps(x, w_qkv, w_out, b_out):
    x = np.asarray(x, dtype=np.float32)
    w_qkv = np.asarray(w_qkv, dtype=np.float32)
    w_out = np.asarray(w_out, dtype=np.float32)
    b_out = np.asarray(b_out, dtype=np.float32)
    zeros = np.zeros_like(b_out)

    xT = [np.ascontiguousarray(x[b].T) for b in range(B)]
    wq = []
    wo = []
    for g in range(2):
        cols = np.concatenate(
            [w_qkv[:, t * DIM + g * GD: t * DIM + (g + 1) * GD] for t in range(3)],
            axis=1)
        wq.append(np.ascontiguousarray(cols))
        wo.append(np.ascontiguousarray(w_out[g * GD:(g + 1) * GD]))

    in_maps = []
    for c in range(NC):
        b, g = divmod(c, 2)
        in_maps.append({"x": xT[b], "w_qkv": wq[g], "w_out": wo[g],
                        "b_out": b_out if g == 0 else zeros})
    return in_maps


def kernel(x, w_qkv, w_out, b_out):
    from concourse.bass_utils import run_bass_kernel_spmd

    nc = _get_compiled()
    in_maps = _make_in_maps(x, w_qkv, w_out, b_out)
    res = run_bass_kernel_spmd(nc, in_maps, core_ids=list(range(NC)))

    out = np.empty((B, N, DIM), dtype=np.float32)
    for b in range(B):
        out[b] = res.results[2 * b]["y"] + res.results[2 * b + 1]["y"]
    return out


# revision 25
# speedup vs baseline: 1.1924x; 1.0091x over previous
"""Self-contained Bass/Trainium2 kernel for nn_Attention (B=4, N=2048, D=1024, H=16, dh=64).

Sharding: 8 cores = (batch b in 0..3) x (head-group g in 0..1, 8 heads each).
Each core projects q/k/v only for its own 8 heads over the full sequence
(no duplicated projection work), runs attention for those heads over all
2048 rows, and computes a partial output projection against its 512 rows
of w_out. The host sums the two partial outputs per batch (row-parallel
linear reduction) -- no on-chip communication.

Numerics: projections in float32r (TF32-class), q/k/v and attention
weights in fp16, accumulation in fp32 PSUM. Softmax uses exp-sum-divide
without max subtraction (scores are O(1)); row sums come free from a
ones-column appended to V; 1/sum via the fast approximate DVE reciprocal.
The bias is added on even cores only (odd cores receive zeros) so the
host-side pair sum stays correct.
"""

import sys
import numpy as np

sys.path.insert(0, "/opt/trn_rl_repo")

B, N, DIM = 4, 2048, 1024
HEADS, DH = 16, 64
SCALE = DH ** -0.5  # 0.125
NC = 8
HLOC = HEADS // 2        # 8 heads per core
GD = HLOC * DH           # 512 projected dims per core per q/k/v
HP = HLOC // 2           # 4 head pairs
CT = DIM // 128          # 8 contraction tiles over input channels
JT = N // 128            # 16 key tiles
ICH = 512                # i-chunk width for attention
NCH = N // ICH           # 4 chunks
VW = DH + 1              # 65: v block width incl. ones column

_compiled = None


def _build():
    import concourse.tile as tile
    from concourse import bacc, mybir

    f32 = mybir.dt.float32
    f32r = mybir.dt.float32r
    f16 = mybir.dt.float16
    EXP = mybir.ActivationFunctionType.Exp

    nc = bacc.Bacc("TRN2", target_bir_lowering=False, debug=False, num_devices=NC)

    X = nc.dram_tensor("x", (DIM, N), f32r, kind="ExternalInput").ap()
    WQKV = nc.dram_tensor("w_qkv", (DIM, 3 * GD), f32r, kind="ExternalInput").ap()
    WOUT = nc.dram_tensor("w_out", (GD, DIM), f32r, kind="ExternalInput").ap()
    BOUT = nc.dram_tensor("b_out", (DIM,), f32, kind="ExternalInput").ap()
    Y = nc.dram_tensor("y", (N, DIM), f32, kind="ExternalOutput").ap()

    with tile.TileContext(nc) as tc:
        with tc.tile_pool(name="persist", bufs=1) as persist, \
             tc.tile_pool(name="attn", bufs=1) as attn:

            # ---- persistent SBUF tensors ----
            w_sb = [persist.tile([128, 3 * GD], f32r, tag="wsb", bufs=CT,
                                 name=f"wsb{ct}") for ct in range(CT)]
            wo_sb = [persist.tile([128, DIM], f32r, tag="wo", bufs=HP,
                                  name=f"wo{hp}") for hp in range(HP)]
            qT = [persist.tile([128, N], f16, tag="qT", bufs=HP,
                              name=f"qT{m}") for m in range(HP)]
            kT = [persist.tile([128, N], f16, tag="kT", bufs=HP,
                              name=f"kT{m}") for m in range(HP)]
            v_ext = [persist.tile([128, HLOC * VW], f16, tag="vext", bufs=JT,
                                  name=f"vext{t}") for t in range(JT)]

            bias_src = persist.tile([1, DIM], f32, tag="bias_src")
            nc.sync.dma_start(bias_src[:], BOUT.rearrange("(o d) -> o d", o=1))
            bias = persist.tile([128, DIM], f32, tag="bias")
            nc.gpsimd.partition_broadcast(bias[:], bias_src[0:1, :])

            # ones columns of v_ext don't depend on data: set them up front
            for t in range(JT):
                ones_col = v_ext[t].rearrange("p (hh c) -> p hh c", c=VW)[:, :, DH:VW]
                nc.gpsimd.memset(ones_col, 1.0)

            # ================= phase A: q/k/v projections =================
            with tc.tile_pool(name="stage", bufs=1) as stage:
                # ---- input DMA, ordered to match first-use ----
                x_sb = {}

                def load_x(c):
                    x_sb[c] = [None] * CT
                    for ct in range(CT):
                        t = stage.tile([128, ICH], f32r, tag="xsb", bufs=2 * CT,
                                       name=f"x{c}_{ct}")
                        nc.sync.dma_start(
                            t[:], X[ct * 128:(ct + 1) * 128, c * ICH:(c + 1) * ICH])
                        x_sb[c][ct] = t

                def load_w(third, ct):
                    nc.sync.dma_start(
                        w_sb[ct][:, third * GD:(third + 1) * GD],
                        WQKV[ct * 128:(ct + 1) * 128, third * GD:(third + 1) * GD])

                # interleave x chunk 0 with w-q so the first accumulation
                # group streams as its operands land
                x_sb[0] = [None] * CT
                for ct in range(CT):
                    t = stage.tile([128, ICH], f32r, tag="xsb", bufs=2 * CT,
                                   name=f"x0_{ct}")
                    nc.sync.dma_start(t[:], X[ct * 128:(ct + 1) * 128, 0:ICH])
                    x_sb[0][ct] = t
                    load_w(0, ct)
                for ct in range(CT):
                    load_w(1, ct)
                for ct in range(CT):
                    load_w(2, ct)
                load_x(1)

                with tc.tile_pool(name="psA", bufs=1, space="PSUM") as psA:
                    for c in range(NCH):
                        xs = x_sb[c]
                        csl = slice(c * ICH, (c + 1) * ICH)
                        # q then k: psum [128 dims, 512 cols], contract channels
                        for third, dst in ((0, qT), (1, kT)):
                            for m in range(HP):
                                ps = psA.tile([128, ICH], f32, tag="proj", bufs=4,
                                              name=f"ps{third}_{c}_{m}")
                                base = third * GD + m * 128
                                for ct in range(CT):
                                    nc.tensor.matmul(ps[:], w_sb[ct][:, base:base + 128],
                                                     xs[ct][:],
                                                     start=(ct == 0), stop=(ct == CT - 1))
                                nc.vector.tensor_copy(dst[m][:, csl], ps[:])
                        # v directly in [row, dim] layout: x block is stationary
                        for jl in range(ICH // 128):
                            ps = psA.tile([128, GD], f32, tag="proj", bufs=4,
                                          name=f"psv_{c}_{jl}")
                            for ct in range(CT):
                                nc.tensor.matmul(ps[:],
                                                 xs[ct][:, jl * 128:(jl + 1) * 128],
                                                 w_sb[ct][:, 2 * GD:3 * GD],
                                                 start=(ct == 0), stop=(ct == CT - 1))
                            dst = v_ext[c * (ICH // 128) + jl].rearrange(
                                "p (hh c) -> p hh c", c=VW)[:, :, 0:DH]
                            nc.vector.tensor_copy(
                                dst, ps.rearrange("p (hh c) -> p hh c", c=DH))
                        # prefetch x for chunk c+2 only after chunk c's reads
                        # are emitted (slot-reuse WAR dependency needs them)
                        if c + 2 < NCH:
                            load_x(c + 2)
                        if c == 0:  # w_out needed only in phase B
                            for hp in range(HP):
                                nc.sync.dma_start(wo_sb[hp][:],
                                                  WOUT[hp * 128:(hp + 1) * 128, :])

            # ================= phase B: attention + output projection =================
            with tc.tile_pool(name="psB", bufs=1, space="PSUM") as psB:
                ctx = {}

                def make_out_ops(ch):
                    """32 micro-ops computing y rows [ch*512, (ch+1)*512)."""
                    ops = []
                    ypb = {}

                    def mk(ib, ec, hp):
                        def op():
                            if hp == 0:
                                ypb[(ib, ec)] = psB.tile(
                                    [128, 512], f32, tag="yp", bufs=2,
                                    name=f"yp{ch}_{ib}_{ec}")
                            yp = ypb[(ib, ec)]
                            nc.tensor.matmul(
                                yp[:], ctx[(ch, hp)][:, ib * 128:(ib + 1) * 128],
                                wo_sb[hp][:, ec * 512:(ec + 1) * 512],
                                start=(hp == 0), stop=(hp == HP - 1))
                            if hp == HP - 1:
                                ysb = attn.tile([128, 512], f32, tag="ysb", bufs=2,
                                                name=f"ysb{ch}_{ib}_{ec}")
                                nc.vector.tensor_add(
                                    ysb[:], yp[:], bias[:, ec * 512:(ec + 1) * 512])
                                r0 = ch * ICH + ib * 128
                                nc.sync.dma_start(
                                    Y[r0:r0 + 128, ec * 512:(ec + 1) * 512], ysb[:])
                        return op

                    for ib in range(ICH // 128):
                        for ec in range(2):
                            for hp in range(HP):
                                ops.append(mk(ib, ec, hp))
                    return ops

                for ch in range(NCH):
                    isl = slice(ch * ICH, (ch + 1) * ICH)
                    oplist = make_out_ops(ch - 1) if ch > 0 else []
                    k = 0
                    for hp in range(HP):
                        cx = attn.tile([128, ICH], f32r, tag="ctx", bufs=8,
                                       name=f"ctx{ch}_{hp}")
                        ctx[(ch, hp)] = cx
                        po = [psB.tile([65, ICH], f32, tag="po", bufs=2,
                                       name=f"po{ch}_{hp}_{p}") for p in range(2)]
                        ats = {}

                        def av(j):
                            for p in range(2):
                                hd = 2 * hp + p
                                nc.tensor.matmul(
                                    po[p][:],
                                    v_ext[j][:, hd * VW:(hd + 1) * VW],
                                    ats[j][:, p * 512:(p + 1) * 512],
                                    start=(j == 0), stop=(j == JT - 1))

                        for jt in range(JT):
                            pp = psB.tile([128, 1024], f32, tag="dots", bufs=2,
                                          name=f"pp{ch}_{hp}_{jt}")
                            for p in range(2):
                                nc.tensor.matmul(
                                    pp[:, p * 512:(p + 1) * 512],
                                    kT[hp][p * 64:(p + 1) * 64, jt * 128:(jt + 1) * 128],
                                    qT[hp][p * 64:(p + 1) * 64, isl],
                                    start=True, stop=True)
                            at = attn.tile([128, 1024], f16, tag="at", bufs=5,
                                           name=f"at{ch}_{hp}_{jt}")
                            nc.scalar.activation(at[:], pp[:], EXP,
                                                 bias=0.0, scale=SCALE)
                            ats[jt] = at
                            # lag-2 so PE never head-of-line blocks on ACT jitter
                            if jt >= 2:
                                av(jt - 2)
                                del ats[jt - 2]
                            # interleave out-proj only in the hp's back half so
                            # it never waits on the previous chunk's last
                            # normalize chain (still in the DVE pipe early on)
                            if jt >= JT - 8 and k < len(oplist):
                                # bias the scheduler against hoisting these to
                                # the chunk boundary where ctx is still in the
                                # normalize pipe
                                tc.cur_priority += 1000000
                                oplist[k]()
                                tc.cur_priority -= 1000000
                                k += 1
                        av(JT - 2)
                        av(JT - 1)
                        # evacuate po at once (a single cheap copy frees the
                        # PSUM slot); the slow reciprocal chain then runs off
                        # the SBUF copy without gating the next head-pair
                        cxu = []
                        for p in range(2):
                            cu = attn.tile([65, ICH], f32, tag="cxu", bufs=4,
                                           name=f"cxu{ch}_{hp}_{p}")
                            nc.vector.tensor_copy(cu[:], po[p][:])
                            cxu.append(cu)
                        # normalize in 128-col blocks: DVE op time scales with
                        # free size, so the first ctx block is ready ~3x sooner
                        # and the scheduler-hoisted out-proj reads barely wait
                        for b4 in range(4):
                            bsl = slice(b4 * 128, (b4 + 1) * 128)
                            rss = []
                            for p in range(2):
                                rs = attn.tile([1, 128], f32, tag="rs", bufs=4,
                                               name=f"rs{ch}_{hp}_{p}_{b4}")
                                nc.vector.reciprocal(rs[:], cxu[p][64:65, bsl])
                                rss.append(rs)
                            for p in range(2):
                                rb = attn.tile([128, 128], f32, tag="rb", bufs=4,
                                               name=f"rb{ch}_{hp}_{p}_{b4}")
                                nc.gpsimd.partition_broadcast(rb[:], rss[p][0:1, :])
                                nc.vector.tensor_mul(cx[p * 64:(p + 1) * 64, bsl],
                                                     cxu[p][0:64, bsl],
                                                     rb[0:64, :])
                    while k < len(oplist):
                        oplist[k]()
                        k += 1
                # tail: output projection of the last chunk
                for op in make_out_ops(NCH - 1):
                    op()

    nc.compile()
    return nc


def _get_compiled():
    global _compiled
    if _compiled is None:
        _compiled = _build()
    return _compiled


def _make_in_maps(x, w_qkv, w_out, b_out):
    x = np.asarray(x, dtype=np.float32)
    w_qkv = np.asarray(w_qkv, dtype=np.float32)
    w_out = np.asarray(w_out, dtype=np.float32)
    b_out = np.asarray(b_out, dtype=np.float32)
    zeros = np.zeros_like(b_out)

    xT = [np.ascontiguousarray(x[b].T) for b in range(B)]
    wq = []
    wo = []
    for g in range(2):
        cols = np.concatenate(
            [w_qkv[:, t * DIM + g * GD: t * DIM + (g + 1) * GD] for t in range(3)],
            axis=1)
        wq.append(np.ascontiguousarray(cols))
        wo.append(np.ascontiguousarray(w_out[g * GD:(g + 1) * GD]))

    in_maps = []
    for c in range(NC):
        b, g = divmod(c, 2)
        in_maps.append({"x": xT[b], "w_qkv": wq[g], "w_out": wo[g],
                        "b_out": b_out if g == 0 else zeros})
    return in_maps


def kernel(x, w_qkv, w_out, b_out):
    from concourse.bass_utils import run_bass_kernel_spmd

    nc = _get_compiled()
    in_maps = _make_in_maps(x, w_qkv, w_out, b_out)
    res = run_bass_kernel_spmd(nc, in_maps, core_ids=list(range(NC)))

    out = np.empty((B, N, DIM), dtype=np.float32)
    for b in range(B):
        out[b] = res.results[2 * b]["y"] + res.results[2 * b + 1]["y"]
    return out
